# revision 43
# baseline (speedup 1.0000x reference)
"""BigBird attention (faithful .view-split variant) on 8 Trainium2 NeuronCores.

Sharding: the reference's `.reshape(B, H, S, hd)` head-split makes each
(batch, head) attend over a [2048, 64] row-major reshape of a 128-token
chunk's [128, 1024] projection. The 2*16 = 32 (b,h) pairs are sharded 4 per
core (batch x head parallel). The output projection is computed per-core as
a partial sum over its 4 heads (row-parallel over Wo), partials are summed
on the host.

Per core (v3 — ScalarE-exp / PE co-scheduling):
  A) QKV projections in bf16, emitted as half-column "quanta" (8 matmuls
     into one [128,512] psum bank + evacuation) interleaved into phase
     B's group loops as PE filler.  q/k bounce via DRAM with a
     transposed read-back on the sync queue; weight loads are split
     across sync/scalar (wq need-order interleaved), wv rides
     gpsimd/SWDGE behind a WAR-delay so it cannot starve the critical
     early wq slices.  The scalar (ACT) queue carries ONLY head-time
     loads: DMA instructions on it would block the B-phase exps.
  B) Per chunk: block-sparse attention. Score strips packed into
     [128,512] psum group tiles, one wide exp per group on ScalarE
     (scale=1/8 folded), inactive blocks zeroed on GpSimd. AV matmuls in
     bf16 with a ones column on V giving softmax sums for free
     (LOOKAHEAD=3 score runway). Normalization is fully on-chip: the
     psum sums row is cast to a bf16 SBUF row, broadcast to 64
     partitions with a rank-1 PE matmul, reciprocated at base partition
     0 with the single-pass custom-DVE reciprocal_approx_fast (which
     misbehaves on PSUM sources and at partition bases > 0), then
     multiplied into o2 on DVE.  No DRAM broadcast round trips.
  C) Partial output projection y^T = sum_h Wo_h O_h^T, interleaved into
     the LAST B chunk at psum-bank-completion points so phase-C matmuls
     fill the exp-paced PE bubbles.

The block mask (band + global cols 0/31 + 3 random blocks) is known at
trace time from src_blocks/tgt_blocks, so the sparsity plan is specialized
per call.
"""

import numpy as np
import ml_dtypes

import concourse.bass as bass
import concourse.mybir as mybir
import concourse.tile as tile
from concourse import bacc
from concourse.bass_utils import run_bass_kernel_spmd

B, S, DIM = 2, 2048, 1024
NHEADS, HD, BLK = 16, 64, 64
NB = S // BLK          # 32 block rows/cols
NCORES = 8
HPC = NHEADS * B // NCORES  # 4 chunks (b,h) per core
P = 128

f32 = mybir.dt.float32
bf16 = mybir.dt.bfloat16

LAST_EXEC_NS = None
LAST_TRACE = None


def _block_mask(src_blocks, tgt_blocks):
    i = np.arange(NB)[:, None]
    j = np.arange(NB)[None, :]
    bm = (np.abs(i - j) <= 1) | (j == 0) | (j == NB - 1)
    bm[np.asarray(src_blocks), np.asarray(tgt_blocks)] = True
    return bm


def _plan_strips(bm):
    """Cover the active blocks with k-stacked strips.

    Strip = dict(k=[kb...] (1 or 2 k-blocks stacked on partitions),
                 q0, qn (q-block run), act [len(k), qn] bool, kind).
    Active cells are claimed exactly once across strips so softmax sums
    are exact.  The glob strip (columns 0/31, all q) is implicit — it is
    handled separately; this returns band + extra strips only.
    """
    claimed = np.zeros((NB, NB), bool)
    claimed[:, 0] = True
    claimed[:, NB - 1] = True
    strips = []
    # band strips: k-pair (2m-1, 2m), q-blocks [2m-2, 2m+2)
    for m in range(1, NB // 2):
        kbs = [2 * m - 1, 2 * m]
        q0, qn = 2 * m - 2, 4
        act = np.zeros((2, qn), bool)
        for ki, k in enumerate(kbs):
            for qi in range(qn):
                q = q0 + qi
                if bm[q, k] and not claimed[q, k]:
                    act[ki, qi] = True
                    claimed[q, k] = True
        strips.append(dict(k=kbs, q0=q0, qn=qn, act=act, kind="band"))
    # leftover random blocks
    rem = np.argwhere(bm & ~claimed)
    byk = {}
    for q, k in rem:
        byk.setdefault(int(k), []).append(int(q))
    for k, qs in sorted(byk.items()):
        qs = sorted(qs)
        while qs:
            q0 = min(max(qs[0] - 1, 0), NB - 4)
            qn = 4
            act = np.zeros((1, qn), bool)
            rest = []
            for q in qs:
                if q0 <= q < q0 + qn:
                    act[0, q - q0] = True
                    claimed[q, k] = True
                else:
                    rest.append(q)
            qs = rest
            strips.append(dict(k=[k], q0=q0, qn=qn, act=act, kind="extra"))
    return strips


def _plan_groups(strips):
    """Pack strips into [128, 512] psum group tiles (one bank each).

    Group kinds: "glob" (one 512-wide quarter of the global columns) and
    "strip" (up to 2 band/extra strips at 256 cols each; bands and extras
    are never mixed within one group so every matmul into a given psum
    bank writes the same partition range).  Order: glob, extras, bands —
    extras early so psum-bank piece counts complete in qb order.
    """
    groups = [dict(kind="glob", qh=qh) for qh in range(4)]
    bands = sorted([s for s in strips if s["kind"] == "band"],
                   key=lambda s: s["q0"])
    extras = sorted([s for s in strips if s["kind"] == "extra"],
                    key=lambda s: s["q0"])
    for j in range(0, len(extras), 2):
        groups.append(dict(kind="strip", strips=extras[j:j + 2]))
    for j in range(0, len(bands), 2):
        groups.append(dict(kind="strip", strips=bands[j:j + 2]))
    return groups


def _build_program(strips, use_bias=True):
    nc = bacc.Bacc("TRN2", target_bir_lowering=False, debug=False,
                   num_devices=NCORES)

    # ---- per-core external inputs ----
    d_xt = nc.dram_tensor("xt", [HPC, P, DIM], bf16, kind="ExternalInput")
    d_wq = nc.dram_tensor("wq", [P, 8 * DIM], bf16, kind="ExternalInput")
    d_wk = nc.dram_tensor("wk", [P, 8 * DIM], bf16, kind="ExternalInput")
    d_wv = nc.dram_tensor("wv", [P, 8 * DIM], bf16, kind="ExternalInput")
    d_bq = nc.dram_tensor("bq", [1, DIM], f32, kind="ExternalInput")
    d_bk = nc.dram_tensor("bk", [1, DIM], f32, kind="ExternalInput")
    d_bv = nc.dram_tensor("bv", [1, DIM], f32, kind="ExternalInput")
    d_wo = nc.dram_tensor("wo", [2, P, DIM], bf16, kind="ExternalInput")
    # y^T partials, tiled: [qb, m2, p, hh*512+c] = y^T[(2*m2+hh)*128+p,
    # qb*512+c] so each phase-C psum tile flushes with ONE dma
    d_yt = nc.dram_tensor("yt", [4, 4, P, 1024], bf16, kind="ExternalOutput")

    with tile.TileContext(nc) as tc:
        _emit(nc, tc, strips, d_xt, (d_wq, d_wk, d_wv),
              (d_bq, d_bk, d_bv), d_wo, d_yt, use_bias)
    nc.compile()
    return nc


def _emit(nc, tc, strips, d_xt, d_w, d_b, d_wo, d_yt, use_bias):
    from contextlib import ExitStack
    groups = _plan_groups(strips)
    ng = len(groups)
    with ExitStack() as ctx:
        psMM = ctx.enter_context(tc.tile_pool(name="psMM", bufs=4,
                                              space="PSUM"))
        psOT = ctx.enter_context(tc.tile_pool(name="psOT", bufs=4,
                                              space="PSUM"))
        dram = ctx.enter_context(tc.tile_pool(name="dram", bufs=1,
                                              space="DRAM"))
        sbB = ctx.enter_context(tc.tile_pool(name="sbB", bufs=1))
        sbQK = ctx.enter_context(tc.tile_pool(name="sbQK", bufs=1))
        sbV = ctx.enter_context(tc.tile_pool(name="sbV", bufs=1))
        sbN = ctx.enter_context(tc.tile_pool(name="sbN", bufs=2))
        lp = ctx.enter_context(tc.tile_pool(name="lp", bufs=2))
        pe = ctx.enter_context(tc.tile_pool(name="pe", bufs=4))
        yp = ctx.enter_context(tc.tile_pool(name="yp", bufs=3))

        # DRAM scratch: per-chunk projection bounces
        dlin = {}
        for nm, shp in (("q", [S, P]), ("k", [S, P]), ("v", [P, DIM])):
            dlin[nm] = [dram.tile(shp, bf16, tag=f"d{nm}{i}",
                                  name=f"d{nm}{i}")
                        for i in range(HPC)]

        chunk_order = [1, 3, 0, 2]

        xtiles = [sbB.tile([P, DIM], bf16, tag=f"xt{i}", name=f"xt{i}")
                  for i in range(HPC)]
        wq_t = [sbB.tile([P, DIM], bf16, tag=f"wq{kt}", name=f"wq{kt}")
                for kt in range(8)]
        wk_t = [sbB.tile([P, 4 * DIM], bf16, tag=f"wk{hv}", name=f"wk{hv}")
                for hv in range(2)]
        wv_t = [sbB.tile([P, 4 * DIM], bf16, tag=f"wv{hv}", name=f"wv{hv}")
                for hv in range(2)]

        def w_rhs_ap(nm, kt, lo, hi):
            if nm == "q":
                return wq_t[kt][:, lo:hi]
            t = wk_t if nm == "k" else wv_t
            return t[kt // 4][:, (kt % 4) * DIM + lo:(kt % 4) * DIM + hi]

        # Startup loads: wq on the sync queue (with the x tiles it needs
        # interleaved just-in-time), wk on scalar, wv on gpsimd/SWDGE —
        # the three weight streams land in parallel.
        # ALL weight matrices stream as 256KB slices in strict
        # need-order (wq -> wk -> wv), interleaved even/odd across the
        # two HWDGE queues so each projection's kt-th slice lands just
        # before its matmuls consume it.  No queue ever holds a 1MB
        # block that could starve a criticial early slice.
        def wslice(eng, w_t, dsrc, kt):
            if len(w_t) == 8:
                dst = w_t[kt][:]
            else:
                dst = w_t[kt // 4][:, (kt % 4) * DIM:(kt % 4 + 1) * DIM]
            eng.dma_start(dst, dsrc[:, kt * DIM:(kt + 1) * DIM])

        nc.sync.dma_start(xtiles[chunk_order[0]][:], d_xt[chunk_order[0]])
        for kt in range(0, 8, 2):
            wslice(nc.sync, wq_t, d_w[0], kt)
        for kt in range(1, 8, 2):
            wslice(nc.scalar, wq_t, d_w[0], kt)
        nc.sync.dma_start(xtiles[chunk_order[1]][:], d_xt[chunk_order[1]])
        for kt in range(0, 8, 2):
            wslice(nc.sync, wk_t, d_w[1], kt)
        for kt in range(1, 8, 2):
            wslice(nc.scalar, wk_t, d_w[1], kt)
        for kt in range(0, 8, 2):
            wslice(nc.sync, wv_t, d_w[2], kt)
        for kt in range(1, 8, 2):
            wslice(nc.scalar, wv_t, d_w[2], kt)
        for i in chunk_order[2:]:
            nc.sync.dma_start(xtiles[i][:], d_xt[i])
        wob = sbB.tile([P, 2 * DIM], bf16, tag="wob")
        nc.scalar.dma_start(wob[:, 0:DIM], d_wo[0])
        nc.scalar.dma_start(wob[:, DIM:2 * DIM], d_wo[1])

        # Preload the exp ACT table with a dummy activation (overlaps A).
        warm = sbB.tile([1, 8], f32, tag="warm")
        nc.vector.memset(warm[:], 0.0)
        nc.scalar.activation(warm[:], warm[:],
                             mybir.ActivationFunctionType.Exp, scale=1.0)

        # ones row at partition 64 for the rank-1 reciprocal broadcast
        onesb = sbB.tile([65, 64], bf16, tag="onesb")
        nc.gpsimd.memset(onesb[64:65, :], 1.0)

        if use_bias:
            bts = {}
            for nm, db in zip("qkv", d_b):
                bts[nm] = sbB.tile([P, DIM], f32, tag=f"b{nm}")
                nc.scalar.dma_start(bts[nm][:], db[:].to_broadcast((P, DIM)))

        # O2 tiles: head-pair-stacked normalized O^T, consumed by phase C
        o2 = [sbB.tile([P, S], bf16, tag=f"o2_{a}", name=f"o2_{a}")
              for a in range(2)]

        # per-chunk attention-input tiles (filled during phase A)
        qts, kts, ktgs, v2bs, v2gs, vxss = {}, {}, {}, {}, {}, {}
        extras = [s for s in strips if s["kind"] == "extra"]

        # ---------------- Phase A: projection quanta ----------------------
        lints = {}
        pos_ctr = {"q": 0, "k": 0, "v": 0}

        def emit_proj_half(nm, i, half):
            """One quantum: 8 matmuls into one psum bank + evacuation;
            on the second half, the DRAM bounce + dependent loads."""
            ps = psMM.tile([P, 512], f32, tag="mm", name=f"A{nm}{i}h{half}")
            for kt in range(8):
                nc.tensor.matmul(
                    ps[:],
                    lhsT=xtiles[i][:, kt * P:(kt + 1) * P],
                    rhs=w_rhs_ap(nm, kt, half * 512, (half + 1) * 512),
                    start=(kt == 0), stop=(kt == 7))
            if half == 0:
                pos = pos_ctr[nm]
                pos_ctr[nm] += 1
                if nm == "v":
                    lint = lp.tile([P, DIM], bf16, tag="linv",
                                   name=f"lintv{i}")
                else:
                    lint = lp.tile([P, 2 * DIM], bf16, tag=f"lin{nm}",
                                   name=f"lint{nm}{i}")
                    if pos < 2:
                        nc.vector.memset(
                            lint[:].rearrange("p (c x) -> p c x",
                                              x=P)[:, :, 64:P], 0.0)
                lints[(nm, i)] = lint
            lint = lints[(nm, i)]
            if nm == "v":
                sl = slice(half * 512, (half + 1) * 512)
                out_ap = lint[:, sl].rearrange("p (c d) -> p c d", d=64)
                in_ap = ps[:].rearrange("p (c d) -> p c d", d=64)
                if use_bias:
                    nc.vector.tensor_add(
                        out_ap, in_ap,
                        bts[nm][:, sl].rearrange("p (c d) -> p c d", d=64))
                else:
                    nc.vector.tensor_copy(out_ap, in_ap)
            else:
                out_ap = lint[:].rearrange(
                    "p (c x) -> p c x",
                    x=P)[:, half * 8:(half + 1) * 8, 0:64]
                in_ap = ps[:].rearrange("p (c d) -> p c d", d=64)
                if use_bias:
                    nc.vector.tensor_add(
                        out_ap, in_ap,
                        bts[nm][:, half * 512:(half + 1) * 512
                                ].rearrange("p (c d) -> p c d", d=64))
                else:
                    nc.vector.tensor_copy(out_ap, in_ap)
            if half == 0:
                return
            # second half done: bounce out + dependent loads
            if nm in ("q", "k"):
                dt = dlin[nm][i]
                nc.sync.dma_start(dt[:], lint[:])
                tt = sbQK.tile([P, S], bf16, tag=f"{nm}t{i}",
                               name=f"{nm}t{i}")
                nc.sync.dma_start(tt[:], dt[:], transpose=True)
                (qts if nm == "q" else kts)[i] = (tt, 0)
            else:
                nc.gpsimd.dma_start(dlin["v"][i][:], lint[:])
                v2b = sbV.tile([P, 15 * 65], bf16, tag=f"v2b{i}",
                               name=f"v2b{i}")
                nc.sync.dma_start(
                    v2b[:].rearrange("p (g e) -> p g e", e=65)[:, :, 0:64],
                    dlin["v"][i][4:124].rearrange(
                        "(g a) (b d) -> (a b) g d", a=8, d=64))
                v2g = sbV.tile([P, 65], bf16, tag=f"v2g{i}", name=f"v2g{i}")
                nc.sync.dma_start(
                    v2g[0:64, 0:64],
                    dlin["v"][i][0:4].rearrange("t (c d) -> (t c) d", d=64))
                nc.sync.dma_start(
                    v2g[64:128, 0:64],
                    dlin["v"][i][124:128].rearrange("t (c d) -> (t c) d",
                                                    d=64))
                vxs = {}
                for si, st in enumerate(extras):
                    kb = st["k"][0]
                    vx = sbV.tile([64, 65], bf16, tag=f"vx{i}_{si}",
                                  name=f"vx{i}_{si}")
                    nc.sync.dma_start(
                        vx[:, 0:64],
                        dlin["v"][i][kb * 4:kb * 4 + 4].rearrange(
                            "t (c d) -> (t c) d", d=64))
                    vxs[id(st)] = vx
                v2bs[i] = v2b
                v2gs[i] = v2g
                vxss[i] = vxs

        # ---------------- Phase B ----------------------------------------
        class ChunkState:
            def __init__(self, i):
                self.i = i
                self.ot_h = [psOT.tile([65, 512], f32, tag="ot",
                                       name=f"ot{i}_{h}") for h in range(4)]
                self.npieces = [0] * 4
                spans = [(0, S)]
                for st in strips:
                    spans.append((st["q0"] * BLK,
                                  (st["q0"] + st["qn"]) * BLK))
                for qlo, qhi in spans:
                    q = qlo
                    while q < qhi:
                        bk2 = q // 512
                        qe = min(qhi, (bk2 + 1) * 512)
                        self.npieces[bk2] += 1
                        q = qe
                self.done = [0] * 4
                self.pending = []
                self.ssb = sbN.tile([65, S], bf16, tag="ssb",
                                    name=f"ssb{i}")
                self.smf = sbN.tile([64, S], f32, tag="smf",
                                    name=f"smf{i}")
                self.rb = sbN.tile([64, S], f32, tag="rb",
                                   name=f"rb{i}")
                if i % 2 == 1:
                    self.o2t = sbN.tile([64, S], bf16, tag="o2t",
                                        name=f"o2t{i}")

        cstates = {}

        def get_cs(i):
            if i not in cstates:
                cstates[i] = ChunkState(i)
                kt_, kb0 = kts[i]
                ktg = sbV.tile([P, P], bf16, tag=f"ktg{i}", name=f"ktg{i}")
                nc.vector.tensor_copy(
                    ktg[kb0:kb0 + 64, :].rearrange("p (a b) -> p a b",
                                                   b=64),
                    kt_[kb0:kb0 + 64, :].rearrange("p (a b) -> p a b",
                                                   b=64)[:, 0:32:31, :])
                ktgs[i] = ktg
                nc.gpsimd.memset(
                    v2bs[i][:].rearrange("p (g e) -> p g e",
                                         e=65)[:, :, 64:65], 1.0)
                nc.gpsimd.memset(v2gs[i][:, 64:65], 1.0)
                for vx in vxss[i].values():
                    nc.gpsimd.memset(vx[:, 64:65], 1.0)
            return cstates[i]

        def emit_scores(i, gi):
            g = groups[gi]
            cs = get_cs(i)
            qt, qb0 = qts[i]
            kt_, kb0 = kts[i]
            ps = psMM.tile([P, 512], f32, tag="mm", name=f"psB{i}g{gi}")
            if g["kind"] == "glob":
                c0 = g["qh"] * 512
                nc.tensor.matmul(ps[:], lhsT=ktgs[i][kb0:kb0 + 64, :],
                                 rhs=qt[qb0:qb0 + 64, c0:c0 + 512],
                                 start=True, stop=True)
            else:
                sts = g["strips"]
                for idx, st in enumerate(sts):
                    col = idx * 256
                    qlo = st["q0"] * BLK
                    qn = st["qn"] * BLK
                    if st["kind"] == "band":
                        k0 = st["k"][0] * BLK
                        nc.tensor.matmul(
                            ps[:, col:col + qn],
                            lhsT=kt_[kb0:kb0 + 64, k0:k0 + 128],
                            rhs=qt[qb0:qb0 + 64, qlo:qlo + qn],
                            start=(idx == 0), stop=(idx == len(sts) - 1))
                    else:
                        kb = st["k"][0]
                        nc.tensor.matmul(
                            ps[0:64, col:col + qn],
                            lhsT=kt_[kb0:kb0 + 64, kb * BLK:kb * BLK + 64],
                            rhs=qt[qb0:qb0 + 64, qlo:qlo + qn],
                            start=(idx == 0), stop=(idx == len(sts) - 1))
            return ps

        def emit_exp(i, gi, ps):
            g = groups[gi]
            et = pe.tile([P, 512], bf16, tag="eg", name=f"eg{i}_{gi}")
            if g["kind"] == "glob":
                nc.scalar.activation(et[:], ps[:],
                                     mybir.ActivationFunctionType.Exp,
                                     scale=0.125)
            else:
                width = 256 * len(g["strips"])
                rows = P if g["strips"][0]["kind"] == "band" else 64
                nc.scalar.activation(et[0:rows, 0:width], ps[0:rows, 0:width],
                                     mybir.ActivationFunctionType.Exp,
                                     scale=0.125)
                for idx, st in enumerate(g["strips"]):
                    col = idx * 256
                    for ki in range(len(st["k"])):
                        for qi in range(st["qn"]):
                            if not st["act"][ki, qi]:
                                nc.gpsimd.memset(
                                    et[ki * 64:(ki + 1) * 64,
                                       col + qi * 64:col + (qi + 1) * 64],
                                    0.0)
            return et

        def flush_norms(cs, c_hook=None):
            i = cs.i
            a, half = i // 2, i % 2
            while cs.pending:
                bk2 = cs.pending.pop(0)
                sl = slice(bk2 * 512, (bk2 + 1) * 512)
                bc = psMM.tile([P, 512], f32, tag="mm",
                               name=f"bc{i}_{bk2}")
                # broadcast the bf16 sums row to 64 partitions (rank-1),
                # then reciprocal at base 0 (approx_fast misbehaves at
                # partition bases > 0 and on PSUM sources)
                nc.tensor.matmul(bc[0:64, :], lhsT=onesb[64:65, 0:64],
                                 rhs=cs.ssb[64:65, sl],
                                 start=True, stop=True)
                nc.vector.tensor_copy(cs.smf[:, sl], bc[0:64, :])
                nc.vector.reciprocal_approx_fast(cs.rb[:, sl],
                                                 cs.smf[:, sl])
                if half == 0:
                    nc.vector.tensor_mul(o2[a][0:64, sl],
                                         cs.ot_h[bk2][0:64, :],
                                         cs.rb[:, sl])
                else:
                    nc.vector.tensor_mul(cs.o2t[:, sl],
                                         cs.ot_h[bk2][0:64, :],
                                         cs.rb[:, sl])
                    nc.sync.dma_start(o2[a][64:128, sl], cs.o2t[:, sl])
                if c_hook is not None:
                    c_hook(bk2)

        def av_pieces(cs, qlo, qhi, lhs, et, et_col0, rows):
            q = qlo
            while q < qhi:
                bk2 = q // 512
                qe = min(qhi, (bk2 + 1) * 512)
                nc.tensor.matmul(
                    cs.ot_h[bk2][0:65, q - bk2 * 512:qe - bk2 * 512],
                    lhsT=lhs,
                    rhs=et[0:rows, et_col0 + q - qlo:et_col0 + qe - qlo],
                    start=(cs.done[bk2] == 0),
                    stop=(cs.done[bk2] == cs.npieces[bk2] - 1))
                cs.done[bk2] += 1
                if cs.done[bk2] == cs.npieces[bk2]:
                    sl = slice(bk2 * 512, (bk2 + 1) * 512)
                    with nc.allow_low_precision(reason="softmax sums in "
                                                "bf16; ~0.4% rel err "
                                                "within tolerance"):
                        nc.vector.tensor_copy(cs.ssb[64:65, sl],
                                              cs.ot_h[bk2][64:65, :])
                    cs.pending.append(bk2)
                q = qe

        def emit_av(i, gi, et):
            g = groups[gi]
            cs = get_cs(i)
            if g["kind"] == "glob":
                q0 = g["qh"] * 512
                av_pieces(cs, q0, q0 + 512, v2gs[i][:], et, 0, 128)
            else:
                for idx, st in enumerate(g["strips"]):
                    col = idx * 256
                    qlo = st["q0"] * BLK
                    qhi = (st["q0"] + st["qn"]) * BLK
                    if st["kind"] == "band":
                        gidx = (st["k"][0] - 1) // 2
                        av_pieces(cs, qlo, qhi,
                                  v2bs[i][:, gidx * 65:(gidx + 1) * 65],
                                  et, col, 128)
                    else:
                        av_pieces(cs, qlo, qhi, vxss[i][id(st)][:],
                                  et, col, 64)

        # ---------------- Phase C: one qb group ---------------------------
        def emit_C_qb(qb):
            for m2 in range(4):
                pss = [psMM.tile([P, 512], f32, tag="mm",
                                 name=f"psC{qb}{m2}{h}")
                       for h in range(2)]
                for hh in range(2):
                    mt = 2 * m2 + hh
                    for a in range(2):
                        nc.tensor.matmul(
                            pss[hh][:],
                            lhsT=wob[:, a * DIM + mt * P:
                                     a * DIM + (mt + 1) * P],
                            rhs=o2[a][:, qb * 512:(qb + 1) * 512],
                            start=(a == 0), stop=(a == 1))
                yt = yp.tile([P, 1024], bf16, tag="yt")
                nc.vector.tensor_copy(yt[:, 0:512], pss[0][:])
                nc.scalar.copy(yt[:, 512:1024], pss[1][:])
                nc.sync.dma_start(d_yt[qb, m2], yt[:])

        LOOKAHEAD = 3

        def emit_B(i, filler=None, c_hook=None):
            get_cs(i)
            nfill = len(filler) if filler else 0
            step = max(1, ng // (nfill + 1)) if nfill else ng + 1
            fi = 0
            sc_ps = {}
            for t in range(min(LOOKAHEAD, ng)):
                sc_ps[t] = emit_scores(i, t)
            cs = cstates[i]
            for t in range(ng):
                et = emit_exp(i, t, sc_ps.pop(t))
                if t + LOOKAHEAD < ng:
                    sc_ps[t + LOOKAHEAD] = emit_scores(i, t + LOOKAHEAD)
                flush_norms(cs, c_hook)
                if filler and fi < nfill and t % step == step - 1:
                    filler[fi]()
                    fi += 1
                emit_av(i, t, et)
            while filler and fi < nfill:
                filler[fi]()
                fi += 1
            flush_norms(cs, c_hook)

        # ---------------- schedule ----------------------------------------
        c1, c2, c3, c4 = chunk_order  # 1, 3, 0, 2

        # head: q/k/v for the first two chunks (startup-bandwidth paced)
        for nm in "qkv":
            for i in (c1, c2):
                for half in range(2):
                    emit_proj_half(nm, i, half)

        def quanta(nm, i):
            return [lambda nm=nm, i=i, h=h: emit_proj_half(nm, i, h)
                    for h in range(2)]

        # B(1) absorbs chunk c3's projections; B(3) absorbs chunk c4's
        # q/k; B(0) absorbs chunk c4's v; B(2) absorbs phase C.
        emit_B(c1, filler=quanta("q", c3) + quanta("k", c3)
               + quanta("v", c3))
        emit_B(c2, filler=quanta("q", c4) + quanta("k", c4))
        emit_B(c3, filler=quanta("v", c4))
        emit_B(c4, c_hook=emit_C_qb)


def kernel(x, Wq, bq, Wk, bk, Wv, bv, Wo, bo, src_blocks, tgt_blocks,
           _trace=False):
    global LAST_EXEC_NS, LAST_TRACE
    x = np.asarray(x, np.float32)
    bm = _block_mask(np.asarray(src_blocks), np.asarray(tgt_blocks))
    strips = _plan_strips(bm)
    use_bias = bool(np.any(np.asarray(bq)) or np.any(np.asarray(bk))
                    or np.any(np.asarray(bv)))
    nc = _build_program(strips, use_bias)

    # host-side shard prep
    # W layout for rhs: w[p, kt*1024 + j] = W[j, kt*128 + p]
    def w_rhs(W):
        Wt = np.ascontiguousarray(np.asarray(W, np.float32).T)  # [in, out]
        return np.ascontiguousarray(
            Wt.reshape(8, P, DIM).transpose(1, 0, 2).reshape(P, 8 * DIM)
        ).astype(ml_dtypes.bfloat16)

    wq_h, wk_h, wv_h = w_rhs(Wq), w_rhs(Wk), w_rhs(Wv)
    WoT = np.asarray(Wo, np.float32).T  # [in(=64*head), out]
    x4 = x.reshape(B, NHEADS, P, DIM)

    in_maps = []
    for c in range(NCORES):
        b = c // 4
        h0 = 4 * (c % 4)
        xc = x4[b, h0:h0 + 4]                       # [4, 128, 1024]
        xt = np.ascontiguousarray(xc.transpose(0, 2, 1))  # [4, 1024, 128]
        # xt dram layout [4, 128, 8*128]: xts[i, p, kt*128+t] = x[t, kt*128+p]
        xts = np.ascontiguousarray(
            xt.reshape(HPC, 8, P, P).transpose(0, 2, 1, 3).reshape(
                HPC, P, 8 * P)).astype(ml_dtypes.bfloat16)
        wo_c = np.zeros((2, P, DIM), ml_dtypes.bfloat16)
        for a in range(2):
            r0 = 64 * (h0 + 2 * a)
            wo_c[a] = WoT[r0:r0 + 128].astype(ml_dtypes.bfloat16)
        in_maps.append({
            "xt": xts,
            "wq": wq_h, "wk": wk_h, "wv": wv_h,
            "bq": np.asarray(bq, np.float32).reshape(1, DIM),
            "bk": np.asarray(bk, np.float32).reshape(1, DIM),
            "bv": np.asarray(bv, np.float32).reshape(1, DIM),
            "wo": wo_c,
        })

    res = run_bass_kernel_spmd(nc, in_maps, core_ids=list(range(NCORES)),
                               trace=_trace)
    LAST_EXEC_NS = res.exec_time_ns
    LAST_TRACE = (res.instructions_and_trace[1]
                  if res.instructions_and_trace else None)

    y = np.zeros((B, S, DIM), np.float32)
    for c in range(NCORES):
        yt_r = np.asarray(res.results[c]["yt"])  # [qb, m2, p, hh*512+c]
        yT = yt_r.reshape(4, 4, P, 2, 512).transpose(1, 3, 2, 0, 4
                                                     ).reshape(DIM, S)
        y[c // 4] += yT.T.astype(np.float32)
    y += np.asarray(bo, np.float32)
    return y
